# revision 29
# baseline (speedup 1.0000x reference)
"""Segment-mean pooling (AvgPoolingLayer / segment_reduce) on 8 Trainium2 cores.

Strategy
--------
segment_ids are sorted, so each segment occupies a contiguous row range.
Shard rows across 8 cores at segment boundaries (each segment lives on
exactly one core).  Per core, the segment-sum is computed as a chain of
one-hot matmuls on the PE:

    psum[block] += one_hot(ids_tile)^T @ feats_tile

Precision: feats are converted to a single bf16 copy on the host
(2 B/element — max rel err of the segment means ~3e-3, well under the
2e-2 gate), halving HBM traffic vs fp32.  The PE consumes bf16 at
1 cycle/row and accumulates fp32 in PSUM.

Blocks: ONE PSUM block of 128 segments per 4096-row feats chunk, with a
data-chosen (but SPMD-static) start w_k = min over cores of the chunk's
lowest segment.  A chunk spans ~41 segments (+cross-core skew ~30), so
all 32 tiles of the chunk fit its block.  Consecutive blocks overlap by
128 - d_k (d_k = w_{k+1} - w_k ~ 41); the overlap is resolved by one
variable-shift matmul per block: block k's rows [d_k:128) (copied
PSUM->SBUF in bf16 on the otherwise idle Activation engine) are added
into block k+1's rows [0:128-d_k) via a per-block shift weight, built
on the DVE from an iota-vs-(p - d_k) compare.  Block k then outputs
rows [0:d_k) (the last block all 128).

One-hots: ALL 32 slots of a chunk are built in ONE wide DVE
tensor_tensor is_equal, comparing a repeated iota (stride-0 broadcast
over the slot dim) against the per-slot rel values (stride-0 broadcast
over the 128 one-hot columns).  This amortizes the per-instruction DVE
overhead ~32x vs per-slot tensor_scalar ops.  (GpSimd is useless here:
its software tensor_scalar measures ~2.2us per 128x128 tile and
throttles the whole core to half clock.)

DMA layout: rows are assigned to SBUF partitions chunk-wise
(partition p of a 4096-row chunk holds rows [32p, 32p+32)), which makes
every feats DMA a fully linear HBM read with 16 KiB contiguous packets
per partition.  The row permutation is absorbed into the precomputed
rel inputs.

SPMD: one Bass program runs on all 8 cores; all per-core differences
(row windows, relative ids, inverse counts) are carried in the input
data, never in the instruction stream.
"""

import numpy as np
import ml_dtypes

from concourse import bass, mybir, tile
from concourse.bass_utils import run_bass_kernel_spmd

N = 1_000_000
D = 256
S = 10_000
NCORES = 8
P = 128           # rows per matmul tile == SBUF partitions
CHUNK = 32        # tiles per feats DMA == consecutive rows per partition
SPC = S // NCORES # segments owned per core

_f32 = mybir.dt.float32
_bf16 = mybir.dt.bfloat16


def _plan(ids, n_rows, n_cores, segs_per_core, chunk):
    """Host-side plan: per-core row windows + per-chunk block starts.

    Row order is partition-major within each P*chunk-row chunk: tile
    (c, n) covers rows {chunk_start + chunk*p + n : p in 0..P-1}.
    Block k (one per chunk) covers local segments [w[k], w[k]+128).
    Returns (starts, R, w, rel) where w is the [nchunk] block-start
    array and rel the per-core [P, T] relative segment ids (-1 = no
    hit).
    """
    g = np.arange(n_cores + 1, dtype=np.int64) * segs_per_core
    b_rows = np.searchsorted(ids, g, side="left")
    spans = b_rows[1:] - b_rows[:-1]
    R = int(np.ceil(spans.max() / (P * chunk)) * (P * chunk))
    assert R <= n_rows and R >= spans.max()
    starts = np.minimum(b_rows[:-1], n_rows - R)
    T = R // P
    nchunk = T // chunk

    vals = np.stack([ids[s:s + R] for s in starts]).astype(np.int64)
    vals -= g[:-1, None]
    vals_t = vals.reshape(n_cores, nchunk, P, chunk)
    owned = (vals_t >= 0) & (vals_t < segs_per_core)

    w = np.empty(nchunk, dtype=np.int64)
    for c in range(nchunk):
        ok = owned[:, c]
        assert ok.any(), f"chunk {c} has no owned rows on any core"
        lo = int(vals_t[:, c][ok].min())
        hi = int(vals_t[:, c][ok].max())
        assert hi - lo < P, (c, lo, hi)
        w[c] = lo
    assert w[0] == 0 and segs_per_core <= w[-1] + P, (w[0], w[-1])
    assert (np.diff(w) >= 1).all() and (np.diff(w) < P).all(), w

    rel = vals_t - w[None, :, None, None]
    hit = owned & (rel >= 0) & (rel < P)
    assert hit[owned].all()
    # [C, nchunk, P, chunk] -> [C, P, nchunk*chunk] slot order
    rel = np.where(hit, rel, -1).astype(np.float32).transpose(0, 2, 1, 3)
    rel = rel.reshape(n_cores, P, T)
    return starts, R, w, rel


def _build_program(R, d, w, chunk):
    """Emit the SPMD Bass program (identical for all cores)."""
    T = R // P
    nchunk = T // chunk
    nblk = nchunk
    out_rows = int(w[-1]) + P
    nc = bass.Bass()
    hb_d = nc.dram_tensor("hb", [R, d], _bf16, kind="ExternalInput")
    iota_d = nc.dram_tensor("iota", [P, P], _bf16, kind="ExternalInput")
    # each rel value stored twice adjacently: lets the wide one-hot
    # tensor_tensor present stride-1 innermost pairs on every operand,
    # which is the DVE's 2x_1p fast-mode requirement
    relb_d = nc.dram_tensor("relb", [P, 2 * T], _bf16, kind="ExternalInput")
    # nblk inv columns (f32) + nblk-1 shift-scalar columns (p - d_k)
    meta_d = nc.dram_tensor("meta", [P, 2 * nblk - 1], _f32,
                            kind="ExternalInput")
    out_d = nc.dram_tensor("out", [out_rows, d], _f32, kind="ExternalOutput")

    with tile.TileContext(nc) as tc:
        with (
            tc.tile_pool(name="const", bufs=1) as cpool,
            tc.tile_pool(name="feats", bufs=5) as fpool,
            tc.tile_pool(name="oh", bufs=3) as ohpool,
            tc.tile_pool(name="acc", bufs=4, space=bass.MemorySpace.PSUM) as pspool,
            tc.tile_pool(name="cpy", bufs=2) as cppool,
            tc.tile_pool(name="res", bufs=3) as rpool,
        ):
            # PE warm-up FIRST: the memsets depend on nothing, so the PE
            # starts ramping (0.65/1.2 -> 2.4 GHz) at t~0 while the first
            # feats chunk is still in flight.
            warm = cpool.tile([P, P], _bf16, name="warm")
            nc.vector.memset(warm[:], 0.0)
            warm_rhs = cpool.tile([P, d], _bf16, name="warm_rhs")
            nc.vector.memset(warm_rhs[:], 0.0)
            wacc = pspool.tile([P, d], _f32, name="wacc", tag="acc")
            for _ in range(8):
                nc.tensor.matmul(wacc[:], warm[:], warm_rhs[:],
                                 start=True, stop=True)

            # constants FIRST on the sync queue: the 16 DMA engines drain
            # queued descriptors in order, so anything issued after a
            # feats chunk (2 MB) completes several microseconds late —
            # the ~570 KB of constants cost the feats stream only ~1.6us
            iota_tile = cpool.tile([P, P], _bf16)
            nc.sync.dma_start(iota_tile[:], iota_d[:])
            relb_t = cpool.tile([P, 2 * T], _bf16)
            nc.sync.dma_start(relb_t[:], relb_d[:])
            meta_t = cpool.tile([P, 2 * nblk - 1], _f32)
            nc.sync.dma_start(meta_t[:], meta_d[:])

            hl0 = fpool.tile([P, chunk, d], _bf16)
            nc.sync.dma_start(
                hl0[:], hb_d[0:chunk * P].rearrange("(p n) d -> p n d", p=P))
            iota_t = iota_tile[:]
            inv_t = meta_t[:, 0:nblk]
            shsc_t = meta_t[:, nblk:]

            # per-block shift weights (ones at (d_b+m, m); the all-zero
            # high columns make the full-width shift matmul add 0 to the
            # rows beyond the overlap).  Built lazily, one per chunk, so
            # they never delay the first wide one-hot op on the DVE.
            shws = [cpool.tile([P, P], _bf16, name=f"shw{b}")
                    for b in range(nblk - 1)]

            def build_shw(b):
                nc.vector.tensor_scalar(
                    out=shws[b][:], in0=iota_t,
                    scalar1=shsc_t[:, b:b + 1], scalar2=None,
                    op0=mybir.AluOpType.is_equal)

            psum_tiles = {}

            def emit_shift(b, pt):
                # add block b's rows [d_b:128) into block b+1's rows
                # [0:128-d_b): copy PSUM->SBUF bf16 on the Activation
                # engine, then a matmul against the per-block shift
                # weight.
                cp = cppool.tile([P, d], _bf16, name="cpy", tag="cpy")
                nc.scalar.activation(
                    cp[:], pt[:, :], mybir.ActivationFunctionType.Copy)
                nc.tensor.matmul(psum_tiles[b + 1][:, :], shws[b][:], cp[:],
                                 start=False, stop=False,
                                 skip_group_check=True)

            def emit_scale(b, pt, rows):
                # block b fully accumulated: scale by 1/count on the
                # Activation engine (Copy with a per-partition scale
                # operand) so the DVE stays dedicated to one-hot builds.
                res = rpool.tile([P, d], _f32, name="res", tag="res")
                nc.scalar.activation(
                    res[0:rows, :], pt[0:rows, :],
                    mybir.ActivationFunctionType.Copy,
                    scale=inv_t[0:rows, b:b + 1])
                # out-DMA on the (otherwise idle) GpSimd DGE queue: on the
                # Scalar queue it would sit between consecutive Activation
                # copies and stall the shift chain
                nc.gpsimd.dma_start(
                    out_d[int(w[b]):int(w[b]) + rows, :], res[0:rows, :])

            # Defer block b's copy+shift a few tiles into chunk b+1 so
            # the PE doesn't stall on the Activation copy, and so block
            # b+1's first matmul (start=True, zeroing PSUM) precedes the
            # shift-in.
            SHIFT_DELAY = 10

            for c in range(nchunk):
                if c == 0:
                    hl = hl0
                else:
                    hl = fpool.tile([P, chunk, d], _bf16)
                    r0 = c * chunk * P
                    src = hb_d[r0:r0 + chunk * P].rearrange(
                        "(p n) d -> p n d", p=P)
                    nc.sync.dma_start(hl[:], src)
                if c > 0:
                    build_shw(c - 1)
                # this chunk's one-hots in two wide DVE ops (the split
                # halves the latency before the PE can start the chunk).
                # All operands present innermost stride-1 PAIRS (iota and
                # out reshaped to [..., 64, 2]; rel duplicated 2x on the
                # host) to qualify for the DVE 2x_1p fast mode.
                oh = ohpool.tile([P, chunk, P], _bf16)
                half = chunk // 2
                for h in range(2):
                    s0 = c * chunk + h * half
                    nc.vector.tensor_tensor(
                        out=oh[:, h * half:(h + 1) * half, :].rearrange(
                            "p s (a b) -> p s a b", b=2),
                        in0=iota_t[:, None, :].rearrange(
                            "p o (a b) -> p o a b", b=2).broadcast_to(
                            [P, half, P // 2, 2]),
                        in1=relb_t[:, 2 * s0:2 * (s0 + half)].rearrange(
                            "p (s b) -> p s b", b=2)[:, :, None, :].broadcast_to(
                            [P, half, P // 2, 2]),
                        op=mybir.AluOpType.is_equal)
                psum_tiles[c] = pspool.tile([P, d], _f32, name="acc",
                                            tag="acc")
                for j in range(chunk):
                    nc.tensor.matmul(psum_tiles[c][:, :],
                                     oh[:, j, :], hl[:, j, :],
                                     start=(j == 0), stop=(j == chunk - 1),
                                     skip_group_check=True)
                    if c > 0 and j == SHIFT_DELAY:
                        emit_shift(c - 1, psum_tiles[c - 1])
                    if c > 0 and j == SHIFT_DELAY + 2:
                        pt = psum_tiles.pop(c - 1)
                        emit_scale(c - 1, pt, int(w[c] - w[c - 1]))
            emit_scale(nchunk - 1, psum_tiles.pop(nchunk - 1), P)
    _strip_self_waits(nc)
    _legalize_waits(nc)
    return nc


# Compute ops whose ISA structs carry a single sync-wait slot.  Tile's
# pool-slot release join sometimes adds a same-engine WAW/WAR wait on top
# of a cross-engine one; same-engine ordering is already guaranteed by
# in-order execution (Tile records same-engine deps as no-sync edges
# elsewhere), so the self-wait is redundant and safe to drop.
_COMPUTE_OPS = (
    mybir.InstTensorTensor, mybir.InstTensorScalarPtr,
    mybir.InstTensorCopy, mybir.InstActivation, mybir.InstMemset,
    mybir.InstMatmult, mybir.InstLdweights, mybir.InstTensorReduce,
)

_COMPUTE_SEMS = ("PE_", "DVE_", "Pool_", "Activation_", "SP_")


def _strip_self_waits(nc):
    for bb in nc.main_func.blocks:
        for ins in bb.instructions:
            si = ins.sync_info
            if si is None or not si.on_wait:
                continue
            if isinstance(ins, _COMPUTE_OPS):
                eng = str(ins.engine).split(".")[-1]
                kept = [w for w in si.on_wait
                        if not w.ant_name.startswith(eng + "_")]
                if len(kept) != len(si.on_wait):
                    si.on_wait = kept
            elif isinstance(ins, mybir.InstDMACopy) and len(si.on_wait) > 1:
                # A WAW wait on the old writer's DMA queue is implied by the
                # compute-engine wait that gates on the old tile's readers
                # (the readers FIFO-follow a wait on that very queue).
                has_compute = any(
                    w.ant_name.startswith(_COMPUTE_SEMS) for w in si.on_wait)
                if has_compute:
                    kept = [w for w in si.on_wait
                            if not w.ant_name.startswith("DMAHW")]
                    if kept and len(kept) != len(si.on_wait):
                        si.on_wait = kept


def _legalize_waits(nc, maxw=1):
    """The walrus codegen here supports very few sync-wait commands per
    instruction.  Hoist excess waits onto preceding same-engine NoOps —
    engine FIFO order makes this equivalent."""
    for bb in nc.main_func.blocks:
        idx = 0
        while idx < len(bb.instructions):
            ins = bb.instructions[idx]
            si = ins.sync_info
            if si is not None and si.on_wait and len(si.on_wait) > maxw:
                waits = list(si.on_wait)
                si.on_wait = waits[-maxw:]
                for w in waits[:-maxw]:
                    nop = mybir.InstNoOp(
                        name=nc.get_next_instruction_name(),
                        engine=ins.engine,
                        sync_info=mybir.SyncInfo(on_wait=[w], on_update=[]),
                        bass_nofuse=True,
                    )
                    bb.instructions.insert(idx, nop)
                    idx += 1
            idx += 1


def _prepare_inputs(feats, ids, n_cores, segs_per_core, starts, R, w, rel):
    """Per-core input maps: bf16 feats + bf16 rel + f32 meta."""
    n, d = feats.shape
    nblk = len(w)
    counts = np.bincount(ids, minlength=n_cores * segs_per_core).astype(np.float32)
    inv = (1.0 / np.maximum(counts, 1.0)).astype(np.float32)
    inv_pad = np.zeros(n_cores * segs_per_core + int(w[-1]) + P, np.float32)
    inv_pad[:inv.shape[0]] = inv

    hb = feats.astype(ml_dtypes.bfloat16)

    iota = np.broadcast_to(np.arange(P, dtype=np.float32), (P, P))
    d_k = np.diff(w)  # [nblk-1] shift distances
    in_maps = []
    for c in range(n_cores):
        g0 = c * segs_per_core
        inv_c = inv_pad[g0:g0 + int(w[-1]) + P].copy()
        inv_c[segs_per_core:] = 0.0
        meta = np.empty((P, 2 * nblk - 1), np.float32)
        for b in range(nblk):
            meta[:, b] = inv_c[int(w[b]):int(w[b]) + P]
        meta[:, nblk:] = (np.arange(P, dtype=np.float32)[:, None]
                          - d_k[None, :].astype(np.float32))
        relb2 = np.repeat(rel[c], 2, axis=1)  # each slot value twice
        in_maps.append({
            "hb": hb[starts[c]:starts[c] + R],
            "iota": iota.astype(ml_dtypes.bfloat16),
            "relb": relb2.astype(ml_dtypes.bfloat16),
            "meta": meta,
        })
    return in_maps


def _run(feats, ids, trace=False, trace_cores=None):
    n, d = feats.shape
    starts, R, w, rel = _plan(ids, n, NCORES, SPC, CHUNK)
    nc = _build_program(R, d, w, CHUNK)
    in_maps = _prepare_inputs(feats, ids, NCORES, SPC, starts, R, w, rel)
    res = run_bass_kernel_spmd(nc, in_maps, list(range(NCORES)),
                               trace=trace, trace_cores=trace_cores)
    out = np.concatenate(
        [res.results[c]["out"][:SPC] for c in range(NCORES)], axis=0)
    return out, res


def kernel(feats, segment_ids, num_segments):
    feats = np.ascontiguousarray(np.asarray(feats), dtype=np.float32)
    ids = np.asarray(segment_ids).astype(np.int64)
    s = int(num_segments)
    assert feats.shape == (N, D) and ids.shape == (N,) and s == S, (
        "kernel is specialized for feats [1e6, 256], 1e4 segments")
    out, _ = _run(feats, ids)
    return out
